# revision 6
# baseline (speedup 1.0000x reference)
"""Trainium2 Bass kernel for DiscreteResidualQuantization (top-6 gather variant).

  z_q = z_e + RMSNormLinear(h - z_e),  z_e = softmax(-||h-c||^2) @ C

Sharding: data-parallel over the N=8192 tokens across 8 NeuronCores
(T=1024 tokens/core); codebook and MLP params replicated.

Key idea vs the 2-GEMM baseline: the softmax over K=8192 logits (std ~78)
is so peaked that z_e is determined by the top-6 entries per token
(truncation rel err 2.6e-5). So GEMM2 (q @ C, half the PE work) is
replaced by:
  per-512-chunk DVE max8/max_index directly on the pass-1 PSUM tiles,
  an index merge (iota/is_equal lookup trick),
  a 6-row indirect-DMA gather of fp16 codebook rows,
  and a per-token weighted sum (z_e); ACT copies z_e into the MLP's
  PSUM banks so the Linear matmuls add on top (z_q = z_e + x@W'+b).

pass1 computes S[t,k] = 2h.c - |c|^2 in [token, k] layout (stationary =
h^T chunks, moving = C^T chunks) with -|c|^2 preloaded into PSUM by the
ACT engine (matmuls run start=False on top).

Loop structure: tokens in 8 blocks of 128; two phases (blocks 0-4, 5-7),
each sweeping K chunk-outer so C^T streams from HBM once per phase and
needs no SBUF residency. Per-block select/gather/MLP work is split into
partA (no PE ops: merge + gather + wsum + residual/rms chain) and partB
(PE transposes + Linear matmuls + evict) so the in-order PE queue never
stalls waiting on partA's chain; phase-0 parts interleave into phase-1's
rounds (partA at kc=3i, partB at kc=3i+2), phase-1 parts run as a
staggered tail (A5 A6 B5 A7 B6 B7) on the then-idle DVE/gpsimd.
"""

import numpy as np

import concourse.bacc as bacc
import concourse.bass as bass
import concourse.mybir as mybir
import concourse.tile as tile
from concourse.bass_utils import run_bass_kernel_spmd
from concourse.masks import make_identity

F32 = mybir.dt.float32
F16 = mybir.dt.float16
U32 = mybir.dt.uint32
I32 = mybir.dt.int32
AF = mybir.ActivationFunctionType
ALU = mybir.AluOpType

N_CORES = 8
EPS = 1e-8
T = 1024
D = 1024
K = 8192
DC = D // 128   # 8 contraction chunks
KC = K // 512   # 16 k chunks
TSN = T // 128  # 8 token blocks
SPLIT = 5       # phase 0 = blocks 0..4, phase 1 = blocks 5..7
J = 6           # top-J softmax support


def build_nc(stop_after=None):
    nc = bacc.Bacc("TRN2", target_bir_lowering=False, debug=False,
                   num_devices=N_CORES)

    hT16 = nc.dram_tensor("hT16", [DC, 128, T], F16, kind="ExternalInput").ap()
    h_nat = nc.dram_tensor("h_nat", [T, D], F32, kind="ExternalInput").ap()
    cbt16 = nc.dram_tensor("cbt16", [KC, 128, DC, 512], F16,
                           kind="ExternalInput").ap()
    cbrows16 = nc.dram_tensor("cbrows16", [K, D], F16,
                              kind="ExternalInput").ap()
    csq2 = nc.dram_tensor("csq2", [2, K], F16, kind="ExternalInput").ap()
    wt16 = nc.dram_tensor("wt16", [D, D], F16, kind="ExternalInput").ap()
    b_row = nc.dram_tensor("b_row", [1, D], F16, kind="ExternalInput").ap()
    zq = nc.dram_tensor("zq", [T, D], F32, kind="ExternalOutput").ap()

    wt16_tiled = wt16.rearrange("(ic ip) o -> ip ic o", ip=128)

    with tile.TileContext(nc) as tc:
        with (
            tc.tile_pool(name="singles", bufs=1) as singles,
            tc.tile_pool(name="cbtp", bufs=5) as cbtp,
            tc.tile_pool(name="candp", bufs=1) as candp,
            tc.tile_pool(name="gp", bufs=2) as gp,
            tc.tile_pool(name="zep", bufs=2) as zep,
            tc.tile_pool(name="wsp", bufs=2) as wsp,
            tc.tile_pool(name="hp", bufs=3) as hp,
            tc.tile_pool(name="xp", bufs=2) as xp,
            tc.tile_pool(name="xtp", bufs=2) as xtp,
            tc.tile_pool(name="zqp", bufs=2) as zqp,
            tc.tile_pool(name="smalls", bufs=3) as smalls,
            tc.tile_pool(name="ps_p1", bufs=1, space="PSUM") as ps_p1,
            tc.tile_pool(name="ps_mlp", bufs=1, space="PSUM") as ps_mlp,
            tc.tile_pool(name="ps_tr", bufs=1, space="PSUM") as ps_tr,
        ):
            # ---- constants resident in SBUF ----
            ident_f16 = singles.tile([128, 128], F16)
            make_identity(nc, ident_f16)
            iota_i32 = singles.tile([128, 128], I32)
            nc.gpsimd.iota(iota_i32, pattern=[[1, 128]], base=0,
                           channel_multiplier=0)
            iota_bcast = singles.tile([128, 128], F32)
            nc.vector.tensor_copy(iota_bcast, iota_i32)
            off_bcast = singles.tile([128, 128], F32)
            for c in range(KC):
                nc.vector.memset(off_bcast[:, c * 8:(c + 1) * 8],
                                 float(c * 512))
            b_sb = singles.tile([1, D], F16)
            nc.gpsimd.dma_start(out=b_sb, in_=b_row)
            ones_row16 = singles.tile([1, 128], F16)
            nc.vector.memset(ones_row16, 1.0)
            # h^T resident fp16 (stationary tiles) -- loaded first,
            # spread across queues so round 0 starts quickly
            hT = []
            for dc in range(DC):
                t_ = singles.tile([128, T], F16, name=f"hT{dc}")
                eng = (nc.sync, nc.scalar)[dc % 2]
                eng.dma_start(out=t_, in_=hT16[dc])
                hT.append(t_)
            # -|c|^2 bias rows (fp16 hi/lo split), injected into each
            # PSUM tile by a 2-partition ones-matmul at group start
            csq_sb = singles.tile([2, K], F16)
            nc.gpsimd.dma_start(out=csq_sb, in_=csq2)
            ones2 = singles.tile([2, 128], F16)
            nc.vector.memset(ones2, 1.0)
            # W^T fp16 resident
            wle = [
                [singles.tile([128, 512], F16, name=f"wle{ic}_{dh}")
                 for dh in range(2)]
                for ic in range(DC)
            ]
            for ic in range(DC):
                for dh in range(2):
                    nc.gpsimd.dma_start(
                        out=wle[ic][dh],
                        in_=wt16_tiled[:, ic, dh * 512:(dh + 1) * 512],
                    )

            # per-block candidate arrays
            vals = [candp.tile([128, KC * 8], F32, name=f"vals{ts}",
                               tag=f"v{ts}") for ts in range(TSN)]
            idxs = [candp.tile([128, KC * 8], U32, name=f"idxs{ts}",
                               tag=f"i{ts}") for ts in range(TSN)]

            # ---------------- per-block parts ----------------
            def make_parts(ts, tail=False):
                """Fixed engine split: DVE scans/merge, ACT weighted
                multiplies (scale-AP) / Square / x-scale, Pool adds/sub +
                gathers. tail=True: weighted sum accumulates in the MLP
                PSUM banks via PE identity-matmuls (partB1) instead of
                Pool adds, so the post-rounds tail has no serial Pool
                chain. Returns (partA, partB) or (partA, partB1, partB2)
                when tail; partA has no PE ops."""
                st = {}
                jn = 3

                def partA():
                    # prefetch h rows for the residual
                    h_sb = hp.tile([128, D], F32, name="h_sb", tag="h")
                    nc.sync.dma_start(
                        out=h_sb, in_=h_nat[ts * 128:(ts + 1) * 128, :])
                    # ---- merge per-chunk top-8 -> global top-J (DVE) ----
                    idxf = smalls.tile([128, KC * 8], F32, name="idxf",
                                       tag="idxf")
                    nc.vector.tensor_copy(idxf, idxs[ts])
                    nc.vector.tensor_add(idxf, idxf, off_bcast)
                    v8 = smalls.tile([128, 8], F32, name="v8", tag="v8")
                    nc.vector.max(v8, vals[ts])
                    pos = smalls.tile([128, 8], U32, name="pos", tag="pos")
                    nc.vector.max_index(pos, v8, vals[ts])
                    posf = smalls.tile([128, 8], F32, name="posf",
                                       tag="posf")
                    nc.vector.tensor_copy(posf, pos)
                    idxsel = smalls.tile([128, jn], F32, name="idxsel",
                                         tag="idxsel")
                    junk = smalls.tile([128, KC * 8], F32, name="junk",
                                       tag="junk")
                    for j in range(jn):
                        nc.vector.scalar_tensor_tensor(
                            out=junk, in0=iota_bcast,
                            scalar=posf[:, j:j + 1],
                            in1=idxf, op0=ALU.is_equal, op1=ALU.mult,
                            accum_out=idxsel[:, j:j + 1],
                        )
                    idx_u32 = smalls.tile([128, jn], U32,
                                          name="idx_u32", tag="idx_u32")
                    nc.vector.tensor_copy(idx_u32, idxsel)
                    # ---- gather top-J codebook rows (fp16) ----
                    gts = []
                    for j in range(jn):
                        gt = gp.tile([128, D], F16, name=f"g{j}",
                                     tag=f"g{j}")
                        nc.gpsimd.indirect_dma_start(
                            out=gt, out_offset=None,
                            in_=cbrows16,
                            in_offset=bass.IndirectOffsetOnAxis(
                                ap=idx_u32[:, j:j + 1], axis=0),
                        )
                        gts.append(gt)
                    # ---- softmax weights from top-J values ----
                    neg_v0 = smalls.tile([128, 1], F32, name="neg_v0",
                                         tag="neg_v0")
                    nc.vector.tensor_scalar_mul(neg_v0, v8[:, 0:1], -1.0)
                    w8 = smalls.tile([128, jn], F32, name="w8",
                                     tag="w8")
                    lsum = smalls.tile([128, 1], F32, name="lsum",
                                       tag="lsum")
                    nc.scalar.activation(out=w8, in_=v8[:, 0:jn],
                                         func=AF.Exp, bias=neg_v0,
                                         scale=1.0, accum_out=lsum)
                    rcp = smalls.tile([128, 1], F32, name="rcp", tag="rcp")
                    nc.vector.reciprocal(rcp, lsum)
                    wn = smalls.tile([128, jn], F32, name="wn",
                                     tag="wn")
                    nc.vector.tensor_scalar_mul(wn, w8, rcp)
                    # ---- z_e = sum_j wn_j * c_idx_j ----
                    # ACT multiplies (per-partition scale AP, fp16 out);
                    # Pool in-place pairwise adds (plain tensor_tensor).
                    tmps = []
                    for j in range(jn):
                        tj = wsp.tile([128, D], F16, name=f"wt{j}",
                                      tag=f"wt{j}")
                        nc.scalar.activation(out=tj, in_=gts[j],
                                             func=AF.Copy,
                                             scale=wn[:, j:j + 1])
                        tmps.append(tj)
                    st["tmps"] = tmps
                    st["h_sb"] = h_sb
                    if tail:
                        return
                    nc.gpsimd.tensor_tensor(out=tmps[0], in0=tmps[0],
                                            in1=tmps[1], op=ALU.add)
                    z_e = zep.tile([128, D], F32, name="z_e", tag="ze")
                    nc.gpsimd.tensor_tensor(out=z_e, in0=tmps[0],
                                            in1=tmps[2], op=ALU.add)
                    # ---- r = h - z_e ; rms ; x ----
                    nc.gpsimd.tensor_tensor(out=h_sb, in0=h_sb, in1=z_e,
                                            op=ALU.subtract)
                    rsq = smalls.tile([128, 1], F32, name="rsq", tag="rsq")
                    junk2 = smalls.tile([128, D], F32, name="junk2",
                                        tag="junk2")
                    nc.scalar.activation(out=junk2, in_=h_sb,
                                         func=AF.Square, accum_out=rsq)
                    rms = smalls.tile([128, 1], F32, name="rms", tag="rms")
                    nc.scalar.activation(out=rms, in_=rsq, func=AF.Sqrt,
                                         scale=1.0 / D)
                    nc.vector.tensor_scalar_add(rms, rms, EPS)
                    rinv = smalls.tile([128, 1], F32, name="rinv",
                                       tag="rinv")
                    nc.vector.reciprocal(rinv, rms)
                    x_sb = xp.tile([128, D], F16, name="x_sb", tag="x")
                    nc.scalar.activation(out=x_sb, in_=h_sb, func=AF.Copy,
                                         scale=rinv)
                    st["z_e"] = z_e
                    st["x_sb"] = x_sb

                def partB():
                    z_e, x_sb = st["z_e"], st["x_sb"]
                    # ---- x^T via PE transposes ----
                    xT_sb = xtp.tile([128, D], F16, name="xT_sb", tag="xt")
                    for icq in range(2):
                        pstx = ps_tr.tile([128, 512], F16, name="pstx",
                                          tag="tr")
                        for j4 in range(4):
                            ic = icq * 4 + j4
                            nc.tensor.transpose(
                                pstx[:, j4 * 128:(j4 + 1) * 128],
                                x_sb[:, ic * 128:(ic + 1) * 128],
                                ident_f16,
                            )
                        nc.scalar.activation(
                            out=xT_sb[:, icq * 512:(icq + 1) * 512],
                            in_=pstx, func=AF.Copy)
                    # ---- Linear accumulates onto z_e in PSUM; evict ----
                    zq_sb = zqp.tile([128, D], F32, name="zq_sb", tag="zq")
                    for dh in range(2):
                        psm = ps_mlp.tile([128, 512], F32, name="psm",
                                          tag=f"mlp{dh}")
                        sl = slice(dh * 512, (dh + 1) * 512)
                        for ic in range(DC):
                            nc.tensor.matmul(
                                psm, xT_sb[:, ic * 128:(ic + 1) * 128],
                                wle[ic][dh], start=(ic == 0), stop=False,
                            )
                        nc.tensor.matmul(
                            psm, ones_row16, b_sb[:, sl],
                            start=False, stop=True,
                        )
                        nc.vector.tensor_add(zq_sb[:, sl], z_e[:, sl], psm)
                    nc.sync.dma_start(
                        out=zq[ts * 128:(ts + 1) * 128, :], in_=zq_sb)

                def partB1():
                    tmps, h_sb = st["tmps"], st["h_sb"]
                    psms = []
                    for dh in range(2):
                        psm = ps_mlp.tile([128, 512], F32, name="psm",
                                          tag=f"mlp{dh}")
                        sl = slice(dh * 512, (dh + 1) * 512)
                        for j in range(jn):
                            nc.tensor.matmul(
                                psm, ident_f16, tmps[j][:, sl],
                                start=(j == 0), stop=False,
                                skip_group_check=True,
                            )
                        psms.append(psm)
                    st["psms"] = psms
                    # r = h - z_e (z_e in psum); rms; x  -- DVE/ACT
                    for dh in range(2):
                        sl = slice(dh * 512, (dh + 1) * 512)
                        nc.vector.tensor_sub(h_sb[:, sl], h_sb[:, sl],
                                             psms[dh])
                    rsq = smalls.tile([128, 1], F32, name="rsq", tag="rsq")
                    junk2 = smalls.tile([128, D], F32, name="junk2",
                                        tag="junk2")
                    nc.scalar.activation(out=junk2, in_=h_sb,
                                         func=AF.Square, accum_out=rsq)
                    rms = smalls.tile([128, 1], F32, name="rms", tag="rms")
                    nc.scalar.activation(out=rms, in_=rsq, func=AF.Sqrt,
                                         scale=1.0 / D)
                    nc.vector.tensor_scalar_add(rms, rms, EPS)
                    rinv = smalls.tile([128, 1], F32, name="rinv",
                                       tag="rinv")
                    nc.vector.reciprocal(rinv, rms)
                    x_sb = xp.tile([128, D], F16, name="x_sb", tag="x")
                    nc.scalar.activation(out=x_sb, in_=h_sb, func=AF.Copy,
                                         scale=rinv)
                    st["x_sb"] = x_sb

                def partB2():
                    x_sb, psms = st["x_sb"], st["psms"]
                    xT_sb = xtp.tile([128, D], F16, name="xT_sb", tag="xt")
                    for icq in range(2):
                        pstx = ps_tr.tile([128, 512], F16, name="pstx",
                                          tag="tr")
                        for j4 in range(4):
                            ic = icq * 4 + j4
                            nc.tensor.transpose(
                                pstx[:, j4 * 128:(j4 + 1) * 128],
                                x_sb[:, ic * 128:(ic + 1) * 128],
                                ident_f16,
                            )
                        nc.scalar.activation(
                            out=xT_sb[:, icq * 512:(icq + 1) * 512],
                            in_=pstx, func=AF.Copy)
                    zq_sb = zqp.tile([128, D], F32, name="zq_sb", tag="zq")
                    for dh in range(2):
                        psm = psms[dh]
                        sl = slice(dh * 512, (dh + 1) * 512)
                        for ic in range(DC):
                            nc.tensor.matmul(
                                psm, xT_sb[:, ic * 128:(ic + 1) * 128],
                                wle[ic][dh], start=False, stop=False,
                                skip_group_check=True,
                            )
                        nc.tensor.matmul(
                            psm, ones_row16, b_sb[:, sl],
                            start=False, stop=True,
                            skip_group_check=True,
                        )
                        nc.scalar.activation(out=zq_sb[:, sl], in_=psm,
                                             func=AF.Copy)
                    nc.sync.dma_start(
                        out=zq[ts * 128:(ts + 1) * 128, :], in_=zq_sb)

                if tail:
                    return partA, partB1, partB2
                return partA, partB

            # ---------------- main phases ----------------
            phases = [list(range(SPLIT)), list(range(SPLIT, TSN))]
            pending = []
            for ph, ts_list in enumerate(phases):
                schedA = {3 * i: i for i in range(SPLIT)}
                schedB = {min(3 * i + 5, KC - 1): i for i in range(SPLIT)}
                for kc in range(KC):
                    cbt_sb = cbtp.tile([128, DC, 512], F16,
                                       name="cbt_sb", tag="cbt")
                    eng = nc.sync if kc % 2 == 0 else nc.scalar
                    eng.dma_start(out=cbt_sb, in_=cbt16[kc])
                    psts = {}
                    for i, ts in enumerate(ts_list):
                        pst = ps_p1.tile([128, 512], F32, name="pst",
                                         tag=f"p1_{i}")
                        psts[ts] = pst
                        nc.tensor.matmul(
                            pst, ones2,
                            csq_sb[:, kc * 512:(kc + 1) * 512],
                            start=True, stop=False,
                            skip_group_check=True)
                        for dc in range(DC):
                            nc.tensor.matmul(
                                pst,
                                hT[dc][:, ts * 128:(ts + 1) * 128],
                                cbt_sb[:, dc, :],
                                start=False, stop=(dc == DC - 1),
                                skip_group_check=True,
                            )
                    for ts in ts_list:
                        nc.vector.max(vals[ts][:, kc * 8:(kc + 1) * 8],
                                      psts[ts])
                        nc.vector.max_index(
                            idxs[ts][:, kc * 8:(kc + 1) * 8],
                            vals[ts][:, kc * 8:(kc + 1) * 8], psts[ts])
                    if ph == 1:
                        if kc in schedA and schedA[kc] < len(pending):
                            pending[schedA[kc]][0]()
                        if kc in schedB and schedB[kc] < len(pending):
                            pending[schedB[kc]][1]()
                if ph == 0:
                    pending = [make_parts(ts) for ts in ts_list]
                    if stop_after == "cand":
                        nc.sync.dma_start(out=zq[0:128, 0:KC * 8],
                                          in_=vals[0])
                        idxf0 = smalls.tile([128, KC * 8], F32,
                                            name="idxf0", tag="idxf")
                        nc.vector.tensor_copy(idxf0, idxs[0])
                        nc.sync.dma_start(out=zq[128:256, 0:KC * 8],
                                          in_=idxf0)
                        break
                else:
                    # tail: gather/mult chains first, then PE parts staggered
                    tails = [make_parts(ts, tail=True) for ts in ts_list]
                    tails[0][0]()
                    tails[1][0]()
                    tails[2][0]()
                    tails[0][1]()
                    tails[1][1]()
                    tails[0][2]()
                    tails[2][1]()
                    tails[1][2]()
                    tails[2][2]()

    nc.compile()
    return nc


def prep_inputs(h, codebook, scale, W, b, n_cores=N_CORES):
    """Host-side reshapes/transposes/casts -> per-core in_maps."""
    h = np.asarray(h, dtype=np.float32)
    codebook = np.ascontiguousarray(np.asarray(codebook, dtype=np.float32))
    scale = np.asarray(scale, dtype=np.float32)
    W = np.asarray(W, dtype=np.float32)
    b = np.asarray(b, dtype=np.float32)

    Tc = h.shape[0] // n_cores
    cb16 = codebook.astype(np.float16)
    # cbt16[kc, p, dc, j] = C^T[dc*128 + p, kc*512 + j]
    cbT = cb16.T  # [D, K]
    cbt16 = np.ascontiguousarray(
        cbT.reshape(DC, 128, KC, 512).transpose(2, 1, 0, 3)
    )
    csq = -np.sum(codebook.astype(np.float64) ** 2, axis=1)
    csq_hi = csq.astype(np.float16)
    csq_lo = (csq - csq_hi.astype(np.float64)).astype(np.float16)
    csq2 = np.ascontiguousarray(np.stack([csq_hi, csq_lo]))
    wt16 = np.ascontiguousarray((W * scale[None, :]).T.astype(np.float16))
    b_row = np.ascontiguousarray(b.reshape(1, D).astype(np.float16))

    in_maps = []
    for c in range(n_cores):
        hc = np.ascontiguousarray(h[c * Tc:(c + 1) * Tc])
        # factor 2 of the cross term baked into h^T (logits = 2h.c - |c|^2)
        hT16 = np.ascontiguousarray(
            (2.0 * hc).T.astype(np.float16).reshape(DC, 128, Tc)
        )
        in_maps.append({
            "hT16": hT16,
            "h_nat": hc,
            "cbt16": cbt16,
            "cbrows16": cb16,
            "csq2": csq2,
            "wt16": wt16,
            "b_row": b_row,
        })
    return in_maps


_NC_CACHE = {}


def get_nc():
    if "nc" not in _NC_CACHE:
        _NC_CACHE["nc"] = build_nc()
    return _NC_CACHE["nc"]


def kernel(h, codebook, scale, W, b):
    nc = get_nc()
    in_maps = prep_inputs(h, codebook, scale, W, b)
    res = run_bass_kernel_spmd(nc, in_maps, core_ids=list(range(N_CORES)))
    out = np.concatenate([r["zq"] for r in res.results], axis=0)
    return out.astype(np.float32)


# revision 7
# speedup vs baseline: 1.0319x; 1.0319x over previous
"""Trainium2 Bass kernel for DiscreteResidualQuantization (top-J gather variant, J=3).

  z_q = z_e + RMSNormLinear(h - z_e),  z_e = softmax(-||h-c||^2) @ C

Sharding: data-parallel over the N=8192 tokens across 8 NeuronCores
(T=1024 tokens/core); codebook and MLP params replicated.

Key idea vs the 2-GEMM baseline: the softmax over K=8192 logits (std ~78)
is so peaked that z_e is determined by the top few entries per
token (J=3 truncation rel err 5.3e-3, well under the 2e-2 budget). So GEMM2 (q @ C, half the PE work) is
replaced by:
  per-512-chunk DVE max8/max_index directly on the pass-1 PSUM tiles,
  an index merge (iota/is_equal lookup trick),
  a J-row indirect-DMA gather of fp16 codebook rows,
  and a per-token weighted sum (z_e); ACT copies z_e into the MLP's
  PSUM banks so the Linear matmuls add on top (z_q = z_e + x@W'+b).

pass1 computes S[t,k] = 2h.c - |c|^2 in [token, k] layout (stationary =
h^T chunks, moving = C^T chunks) with -|c|^2 preloaded into PSUM by the
ACT engine (matmuls run start=False on top).

Loop structure: tokens in 8 blocks of 128; two phases (blocks 0-4, 5-7),
each sweeping K chunk-outer so C^T streams from HBM once per phase and
needs no SBUF residency. Per-block select/gather/MLP work is split into
partA (no PE ops: merge + gather + wsum + residual/rms chain) and partB
(PE transposes + Linear matmuls + evict) so the in-order PE queue never
stalls waiting on partA's chain; phase-0 parts interleave into phase-1's
rounds (partA at kc=3i, partB at kc=3i+2), phase-1 parts run as a
staggered tail (A5 A6 B5 A7 B6 B7) on the then-idle DVE/gpsimd.
"""

import numpy as np

import concourse.bacc as bacc
import concourse.bass as bass
import concourse.mybir as mybir
import concourse.tile as tile
from concourse.bass_utils import run_bass_kernel_spmd
from concourse.masks import make_identity

F32 = mybir.dt.float32
F16 = mybir.dt.float16
U32 = mybir.dt.uint32
I32 = mybir.dt.int32
AF = mybir.ActivationFunctionType
ALU = mybir.AluOpType

N_CORES = 8
EPS = 1e-8
T = 1024
D = 1024
K = 8192
DC = D // 128   # 8 contraction chunks
KC = K // 512   # 16 k chunks
TSN = T // 128  # 8 token blocks
SPLIT = 5       # phase 0 = blocks 0..4, phase 1 = blocks 5..7
J = 8           # per-chunk candidate width (max8); gathers use jn=3


def build_nc(stop_after=None):
    nc = bacc.Bacc("TRN2", target_bir_lowering=False, debug=False,
                   num_devices=N_CORES)

    hT16 = nc.dram_tensor("hT16", [DC, 128, T], F16, kind="ExternalInput").ap()
    h_nat = nc.dram_tensor("h_nat", [T, D], F32, kind="ExternalInput").ap()
    cbt16 = nc.dram_tensor("cbt16", [KC, 128, DC, 512], F16,
                           kind="ExternalInput").ap()
    cbrows16 = nc.dram_tensor("cbrows16", [K, D], F16,
                              kind="ExternalInput").ap()
    csq2 = nc.dram_tensor("csq2", [2, K], F16, kind="ExternalInput").ap()
    wt16 = nc.dram_tensor("wt16", [D, D], F16, kind="ExternalInput").ap()
    b_row = nc.dram_tensor("b_row", [1, D], F16, kind="ExternalInput").ap()
    zq = nc.dram_tensor("zq", [T, D], F32, kind="ExternalOutput").ap()

    wt16_tiled = wt16.rearrange("(ic ip) o -> ip ic o", ip=128)

    with tile.TileContext(nc) as tc:
        with (
            tc.tile_pool(name="singles", bufs=1) as singles,
            tc.tile_pool(name="cbtp", bufs=5) as cbtp,
            tc.tile_pool(name="candp", bufs=1) as candp,
            tc.tile_pool(name="gp", bufs=2) as gp,
            tc.tile_pool(name="zep", bufs=2) as zep,
            tc.tile_pool(name="wsp", bufs=2) as wsp,
            tc.tile_pool(name="hp", bufs=3) as hp,
            tc.tile_pool(name="xp", bufs=2) as xp,
            tc.tile_pool(name="xtp", bufs=2) as xtp,
            tc.tile_pool(name="zqp", bufs=2) as zqp,
            tc.tile_pool(name="smalls", bufs=3) as smalls,
            tc.tile_pool(name="ps_p1", bufs=1, space="PSUM") as ps_p1,
            tc.tile_pool(name="ps_mlp", bufs=1, space="PSUM") as ps_mlp,
            tc.tile_pool(name="ps_tr", bufs=1, space="PSUM") as ps_tr,
        ):
            # ---- constants resident in SBUF ----
            ident_f16 = singles.tile([128, 128], F16)
            make_identity(nc, ident_f16)
            iota_i32 = singles.tile([128, 128], I32)
            nc.gpsimd.iota(iota_i32, pattern=[[1, 128]], base=0,
                           channel_multiplier=0)
            iota_bcast = singles.tile([128, 128], F32)
            nc.vector.tensor_copy(iota_bcast, iota_i32)
            off_bcast = singles.tile([128, 128], F32)
            for c in range(KC):
                nc.vector.memset(off_bcast[:, c * 8:(c + 1) * 8],
                                 float(c * 512))
            b_sb = singles.tile([1, D], F16)
            nc.gpsimd.dma_start(out=b_sb, in_=b_row)
            ones_row16 = singles.tile([1, 128], F16)
            nc.vector.memset(ones_row16, 1.0)
            # h^T resident fp16 (stationary tiles) -- loaded first,
            # spread across queues so round 0 starts quickly
            hT = []
            for dc in range(DC):
                t_ = singles.tile([128, T], F16, name=f"hT{dc}")
                eng = (nc.sync, nc.scalar)[dc % 2]
                eng.dma_start(out=t_, in_=hT16[dc])
                hT.append(t_)
            # -|c|^2 bias rows (fp16 hi/lo split), injected into each
            # PSUM tile by a 2-partition ones-matmul at group start
            csq_sb = singles.tile([2, K], F16)
            nc.gpsimd.dma_start(out=csq_sb, in_=csq2)
            ones2 = singles.tile([2, 128], F16)
            nc.vector.memset(ones2, 1.0)
            # W^T fp16 resident
            wle = [
                [singles.tile([128, 512], F16, name=f"wle{ic}_{dh}")
                 for dh in range(2)]
                for ic in range(DC)
            ]
            for ic in range(DC):
                for dh in range(2):
                    nc.gpsimd.dma_start(
                        out=wle[ic][dh],
                        in_=wt16_tiled[:, ic, dh * 512:(dh + 1) * 512],
                    )

            # per-block candidate arrays
            vals = [candp.tile([128, KC * 8], F32, name=f"vals{ts}",
                               tag=f"v{ts}") for ts in range(TSN)]
            idxs = [candp.tile([128, KC * 8], U32, name=f"idxs{ts}",
                               tag=f"i{ts}") for ts in range(TSN)]

            # ---------------- per-block parts ----------------
            def make_parts(ts, tail=False):
                """Fixed engine split: DVE scans/merge, ACT weighted
                multiplies (scale-AP) / Square / x-scale, Pool adds/sub +
                gathers. tail=True: weighted sum accumulates in the MLP
                PSUM banks via PE identity-matmuls (partB1) instead of
                Pool adds, so the post-rounds tail has no serial Pool
                chain. Returns (partA, partB) or (partA, partB1, partB2)
                when tail; partA has no PE ops."""
                st = {}
                jn = 3

                def partA():
                    # prefetch h rows for the residual
                    h_sb = hp.tile([128, D], F32, name="h_sb", tag="h")
                    nc.sync.dma_start(
                        out=h_sb, in_=h_nat[ts * 128:(ts + 1) * 128, :])
                    # ---- merge per-chunk top-8 -> global top-J (DVE) ----
                    idxf = smalls.tile([128, KC * 8], F32, name="idxf",
                                       tag="idxf")
                    nc.vector.tensor_copy(idxf, idxs[ts])
                    nc.vector.tensor_add(idxf, idxf, off_bcast)
                    v8 = smalls.tile([128, 8], F32, name="v8", tag="v8")
                    nc.vector.max(v8, vals[ts])
                    pos = smalls.tile([128, 8], U32, name="pos", tag="pos")
                    nc.vector.max_index(pos, v8, vals[ts])
                    posf = smalls.tile([128, 8], F32, name="posf",
                                       tag="posf")
                    nc.vector.tensor_copy(posf, pos)
                    idxsel = smalls.tile([128, jn], F32, name="idxsel",
                                         tag="idxsel")
                    junk = smalls.tile([128, KC * 8], F32, name="junk",
                                       tag="junk")
                    for j in range(jn):
                        nc.vector.scalar_tensor_tensor(
                            out=junk, in0=iota_bcast,
                            scalar=posf[:, j:j + 1],
                            in1=idxf, op0=ALU.is_equal, op1=ALU.mult,
                            accum_out=idxsel[:, j:j + 1],
                        )
                    idx_u32 = smalls.tile([128, jn], U32,
                                          name="idx_u32", tag="idx_u32")
                    nc.vector.tensor_copy(idx_u32, idxsel)
                    # ---- gather top-J codebook rows (fp16) ----
                    gts = []
                    for j in range(jn):
                        gt = gp.tile([128, D], F16, name=f"g{j}",
                                     tag=f"g{j}")
                        nc.gpsimd.indirect_dma_start(
                            out=gt, out_offset=None,
                            in_=cbrows16,
                            in_offset=bass.IndirectOffsetOnAxis(
                                ap=idx_u32[:, j:j + 1], axis=0),
                        )
                        gts.append(gt)
                    # ---- softmax weights from top-J values ----
                    neg_v0 = smalls.tile([128, 1], F32, name="neg_v0",
                                         tag="neg_v0")
                    nc.vector.tensor_scalar_mul(neg_v0, v8[:, 0:1], -1.0)
                    w8 = smalls.tile([128, jn], F32, name="w8",
                                     tag="w8")
                    lsum = smalls.tile([128, 1], F32, name="lsum",
                                       tag="lsum")
                    nc.scalar.activation(out=w8, in_=v8[:, 0:jn],
                                         func=AF.Exp, bias=neg_v0,
                                         scale=1.0, accum_out=lsum)
                    rcp = smalls.tile([128, 1], F32, name="rcp", tag="rcp")
                    nc.vector.reciprocal(rcp, lsum)
                    wn = smalls.tile([128, jn], F32, name="wn",
                                     tag="wn")
                    nc.vector.tensor_scalar_mul(wn, w8, rcp)
                    # ---- z_e = sum_j wn_j * c_idx_j ----
                    # ACT multiplies (per-partition scale AP, fp16 out);
                    # Pool in-place pairwise adds (plain tensor_tensor).
                    tmps = []
                    for j in range(jn):
                        tj = wsp.tile([128, D], F16, name=f"wt{j}",
                                      tag=f"wt{j}")
                        nc.scalar.activation(out=tj, in_=gts[j],
                                             func=AF.Copy,
                                             scale=wn[:, j:j + 1])
                        tmps.append(tj)
                    st["tmps"] = tmps
                    st["h_sb"] = h_sb
                    if tail:
                        return
                    nc.gpsimd.tensor_tensor(out=tmps[0], in0=tmps[0],
                                            in1=tmps[1], op=ALU.add)
                    z_e = zep.tile([128, D], F32, name="z_e", tag="ze")
                    nc.gpsimd.tensor_tensor(out=z_e, in0=tmps[0],
                                            in1=tmps[2], op=ALU.add)
                    # ---- r = h - z_e ; rms ; x ----
                    nc.gpsimd.tensor_tensor(out=h_sb, in0=h_sb, in1=z_e,
                                            op=ALU.subtract)
                    rsq = smalls.tile([128, 1], F32, name="rsq", tag="rsq")
                    junk2 = smalls.tile([128, D], F32, name="junk2",
                                        tag="junk2")
                    nc.scalar.activation(out=junk2, in_=h_sb,
                                         func=AF.Square, accum_out=rsq)
                    rms = smalls.tile([128, 1], F32, name="rms", tag="rms")
                    nc.scalar.activation(out=rms, in_=rsq, func=AF.Sqrt,
                                         scale=1.0 / D)
                    nc.vector.tensor_scalar_add(rms, rms, EPS)
                    rinv = smalls.tile([128, 1], F32, name="rinv",
                                       tag="rinv")
                    nc.vector.reciprocal(rinv, rms)
                    x_sb = xp.tile([128, D], F16, name="x_sb", tag="x")
                    nc.scalar.activation(out=x_sb, in_=h_sb, func=AF.Copy,
                                         scale=rinv)
                    st["z_e"] = z_e
                    st["x_sb"] = x_sb

                def partB():
                    z_e, x_sb = st["z_e"], st["x_sb"]
                    # ---- x^T via PE transposes ----
                    xT_sb = xtp.tile([128, D], F16, name="xT_sb", tag="xt")
                    for icq in range(2):
                        pstx = ps_tr.tile([128, 512], F16, name="pstx",
                                          tag="tr")
                        for j4 in range(4):
                            ic = icq * 4 + j4
                            nc.tensor.transpose(
                                pstx[:, j4 * 128:(j4 + 1) * 128],
                                x_sb[:, ic * 128:(ic + 1) * 128],
                                ident_f16,
                            )
                        nc.scalar.activation(
                            out=xT_sb[:, icq * 512:(icq + 1) * 512],
                            in_=pstx, func=AF.Copy)
                    # ---- Linear accumulates onto z_e in PSUM; evict ----
                    zq_sb = zqp.tile([128, D], F32, name="zq_sb", tag="zq")
                    for dh in range(2):
                        psm = ps_mlp.tile([128, 512], F32, name="psm",
                                          tag=f"mlp{dh}")
                        sl = slice(dh * 512, (dh + 1) * 512)
                        for ic in range(DC):
                            nc.tensor.matmul(
                                psm, xT_sb[:, ic * 128:(ic + 1) * 128],
                                wle[ic][dh], start=(ic == 0), stop=False,
                            )
                        nc.tensor.matmul(
                            psm, ones_row16, b_sb[:, sl],
                            start=False, stop=True,
                        )
                        nc.vector.tensor_add(zq_sb[:, sl], z_e[:, sl], psm)
                    nc.sync.dma_start(
                        out=zq[ts * 128:(ts + 1) * 128, :], in_=zq_sb)

                def partB1():
                    tmps, h_sb = st["tmps"], st["h_sb"]
                    psms = []
                    for dh in range(2):
                        psm = ps_mlp.tile([128, 512], F32, name="psm",
                                          tag=f"mlp{dh}")
                        sl = slice(dh * 512, (dh + 1) * 512)
                        for j in range(jn):
                            nc.tensor.matmul(
                                psm, ident_f16, tmps[j][:, sl],
                                start=(j == 0), stop=False,
                                skip_group_check=True,
                            )
                        psms.append(psm)
                    st["psms"] = psms
                    # r = h - z_e (z_e in psum); rms; x  -- DVE/ACT
                    for dh in range(2):
                        sl = slice(dh * 512, (dh + 1) * 512)
                        nc.vector.tensor_sub(h_sb[:, sl], h_sb[:, sl],
                                             psms[dh])
                    rsq = smalls.tile([128, 1], F32, name="rsq", tag="rsq")
                    junk2 = smalls.tile([128, D], F32, name="junk2",
                                        tag="junk2")
                    nc.scalar.activation(out=junk2, in_=h_sb,
                                         func=AF.Square, accum_out=rsq)
                    rms = smalls.tile([128, 1], F32, name="rms", tag="rms")
                    nc.scalar.activation(out=rms, in_=rsq, func=AF.Sqrt,
                                         scale=1.0 / D)
                    nc.vector.tensor_scalar_add(rms, rms, EPS)
                    rinv = smalls.tile([128, 1], F32, name="rinv",
                                       tag="rinv")
                    nc.vector.reciprocal(rinv, rms)
                    x_sb = xp.tile([128, D], F16, name="x_sb", tag="x")
                    nc.scalar.activation(out=x_sb, in_=h_sb, func=AF.Copy,
                                         scale=rinv)
                    st["x_sb"] = x_sb

                def partB2():
                    x_sb, psms = st["x_sb"], st["psms"]
                    xT_sb = xtp.tile([128, D], F16, name="xT_sb", tag="xt")
                    for icq in range(2):
                        pstx = ps_tr.tile([128, 512], F16, name="pstx",
                                          tag="tr")
                        for j4 in range(4):
                            ic = icq * 4 + j4
                            nc.tensor.transpose(
                                pstx[:, j4 * 128:(j4 + 1) * 128],
                                x_sb[:, ic * 128:(ic + 1) * 128],
                                ident_f16,
                            )
                        nc.scalar.activation(
                            out=xT_sb[:, icq * 512:(icq + 1) * 512],
                            in_=pstx, func=AF.Copy)
                    zq_sb = zqp.tile([128, D], F32, name="zq_sb", tag="zq")
                    for dh in range(2):
                        psm = psms[dh]
                        sl = slice(dh * 512, (dh + 1) * 512)
                        for ic in range(DC):
                            nc.tensor.matmul(
                                psm, xT_sb[:, ic * 128:(ic + 1) * 128],
                                wle[ic][dh], start=False, stop=False,
                                skip_group_check=True,
                            )
                        nc.tensor.matmul(
                            psm, ones_row16, b_sb[:, sl],
                            start=False, stop=True,
                            skip_group_check=True,
                        )
                        nc.scalar.activation(out=zq_sb[:, sl], in_=psm,
                                             func=AF.Copy)
                    nc.sync.dma_start(
                        out=zq[ts * 128:(ts + 1) * 128, :], in_=zq_sb)

                if tail:
                    return partA, partB1, partB2
                return partA, partB

            # ---------------- main phases ----------------
            phases = [list(range(SPLIT)), list(range(SPLIT, TSN))]
            pending = []
            for ph, ts_list in enumerate(phases):
                schedA = {3 * i: i for i in range(SPLIT)}
                schedB = {min(3 * i + 5, KC - 1): i for i in range(SPLIT)}
                for kc in range(KC):
                    cbt_sb = cbtp.tile([128, DC, 512], F16,
                                       name="cbt_sb", tag="cbt")
                    eng = nc.sync if kc % 2 == 0 else nc.scalar
                    eng.dma_start(out=cbt_sb, in_=cbt16[kc])
                    psts = {}
                    for i, ts in enumerate(ts_list):
                        pst = ps_p1.tile([128, 512], F32, name="pst",
                                         tag=f"p1_{i}")
                        psts[ts] = pst
                        nc.tensor.matmul(
                            pst, ones2,
                            csq_sb[:, kc * 512:(kc + 1) * 512],
                            start=True, stop=False,
                            skip_group_check=True)
                        for dc in range(DC):
                            nc.tensor.matmul(
                                pst,
                                hT[dc][:, ts * 128:(ts + 1) * 128],
                                cbt_sb[:, dc, :],
                                start=False, stop=(dc == DC - 1),
                                skip_group_check=True,
                            )
                    for ts in ts_list:
                        nc.vector.max(vals[ts][:, kc * 8:(kc + 1) * 8],
                                      psts[ts])
                        nc.vector.max_index(
                            idxs[ts][:, kc * 8:(kc + 1) * 8],
                            vals[ts][:, kc * 8:(kc + 1) * 8], psts[ts])
                    if ph == 1:
                        if kc in schedA and schedA[kc] < len(pending):
                            pending[schedA[kc]][0]()
                        if kc in schedB and schedB[kc] < len(pending):
                            pending[schedB[kc]][1]()
                if ph == 0:
                    pending = [make_parts(ts) for ts in ts_list]
                    if stop_after == "cand":
                        nc.sync.dma_start(out=zq[0:128, 0:KC * 8],
                                          in_=vals[0])
                        idxf0 = smalls.tile([128, KC * 8], F32,
                                            name="idxf0", tag="idxf")
                        nc.vector.tensor_copy(idxf0, idxs[0])
                        nc.sync.dma_start(out=zq[128:256, 0:KC * 8],
                                          in_=idxf0)
                        break
                else:
                    # tail: gather/mult chains first, then PE parts staggered
                    tails = [make_parts(ts, tail=True) for ts in ts_list]
                    tails[0][0]()
                    tails[1][0]()
                    tails[2][0]()
                    tails[0][1]()
                    tails[1][1]()
                    tails[0][2]()
                    tails[2][1]()
                    tails[1][2]()
                    tails[2][2]()

    nc.compile()
    return nc


def prep_inputs(h, codebook, scale, W, b, n_cores=N_CORES):
    """Host-side reshapes/transposes/casts -> per-core in_maps."""
    h = np.asarray(h, dtype=np.float32)
    codebook = np.ascontiguousarray(np.asarray(codebook, dtype=np.float32))
    scale = np.asarray(scale, dtype=np.float32)
    W = np.asarray(W, dtype=np.float32)
    b = np.asarray(b, dtype=np.float32)

    Tc = h.shape[0] // n_cores
    cb16 = codebook.astype(np.float16)
    # cbt16[kc, p, dc, j] = C^T[dc*128 + p, kc*512 + j]
    cbT = cb16.T  # [D, K]
    cbt16 = np.ascontiguousarray(
        cbT.reshape(DC, 128, KC, 512).transpose(2, 1, 0, 3)
    )
    csq = -np.sum(codebook.astype(np.float64) ** 2, axis=1)
    csq_hi = csq.astype(np.float16)
    csq_lo = (csq - csq_hi.astype(np.float64)).astype(np.float16)
    csq2 = np.ascontiguousarray(np.stack([csq_hi, csq_lo]))
    wt16 = np.ascontiguousarray((W * scale[None, :]).T.astype(np.float16))
    b_row = np.ascontiguousarray(b.reshape(1, D).astype(np.float16))

    in_maps = []
    for c in range(n_cores):
        hc = np.ascontiguousarray(h[c * Tc:(c + 1) * Tc])
        # factor 2 of the cross term baked into h^T (logits = 2h.c - |c|^2)
        hT16 = np.ascontiguousarray(
            (2.0 * hc).T.astype(np.float16).reshape(DC, 128, Tc)
        )
        in_maps.append({
            "hT16": hT16,
            "h_nat": hc,
            "cbt16": cbt16,
            "cbrows16": cb16,
            "csq2": csq2,
            "wt16": wt16,
            "b_row": b_row,
        })
    return in_maps


_NC_CACHE = {}


def get_nc():
    if "nc" not in _NC_CACHE:
        _NC_CACHE["nc"] = build_nc()
    return _NC_CACHE["nc"]


def kernel(h, codebook, scale, W, b):
    nc = get_nc()
    in_maps = prep_inputs(h, codebook, scale, W, b)
    res = run_bass_kernel_spmd(nc, in_maps, core_ids=list(range(N_CORES)))
    out = np.concatenate([r["zq"] for r in res.results], axis=0)
    return out.astype(np.float32)
